# revision 8
# baseline (speedup 1.0000x reference)
# Contrastive loss (L2-distance scores, margin hinge, mean reduction) on 8
# Trainium2 NeuronCores.
#
# total = mean(cost_s) + mean(cost_im) over the [N, N] score matrix
#   scores[i, j] = -||im_i - s_j||;  D_ij = ||im_i - s_j||
#   cost_s  = relu(margin + scores - diag_row)   (diag zeroed)
#   cost_im = relu(margin + scores - diag_col)   (diag zeroed)
#
# Device identity:  relu(a - d) = max(a, d) - d, so with a_i = b_i =
# margin + D_ii:
#   sum(cost_s) + sum(cost_im) = sum_ij [max(a_i, D_ij) + max(b_j, D_ij) - 2 D_ij]
#
# Sharding: rows of the score matrix across 8 cores ([1024, 8192] slab each).
# s (columns) is replicated but column-ROTATED per core so every core's
# diagonal block lands at local columns [0, 1024) -> one static SPMD program.
# The diagonal is zeroed exactly by accumulating -BIG into the PSUM diagonal
# (rank-128 identity matmul): D_ii becomes ~sqrt(2*BIG) >> a, b, so
# max(a, D_ii) + max(b, D_ii) - 2 D_ii = 0 exactly.
#
# Per (m-tile of 128 rows, group of 2048 cols = 4 PSUM banks):
#   PE  : per 512-chunk: 2x K=128 bf16 matmuls (dot) + [diag group only]
#         ident x (-BIG*ident) boost + 1x K=4 fp8 DoubleRow matmul adding a
#         4-term fp8 split of -||s_j||^2/2  ->  PSUM q = dot - s_sq/2 (- BIG
#         on diag)
#   ACT : D = sqrt(-2*q + ||im_i||^2)  [PSUM -> SBUF fp32]
#   DVE : one fused custom op:  out = max(D, a_i) + max(D, b_j + BSHIFT)
#         - 2*D, accum_out[p] = sum(out)   (b is stored bf16 SHIFTED by
#         -BSHIFT so its bf16 ulp is ~16x smaller; a stays fp32)
# Final: reduce the 32 accumulator columns, partition-sum via a ones-matmul,
# DMA the per-core scalar out. Host: sum 8 partials, divide by N^2.
#
# Numerics vs the fp32 reference: bf16 point quantization dominates
# (~4e-5 relative); measured ~9e-5 end to end.

import os

import numpy as np
import ml_dtypes

import concourse.bass as bass
import concourse.tile as tile
from concourse import bacc, mybir
from concourse import bass_utils
from concourse import dve_ops as _dve_ops
from concourse.dve_spec import (
    Spec as _DveSpec,
    Src0,
    Src1,
    C0,
    C1,
    C2 as _C2,
    lower as _dve_lower,
    maxx as _dve_maxx,
)
from concourse.dve_uop import DveOpSpec as _DveOpSpec

N = 8192
D = 256
MARGIN = 0.2
NCORES = 8
SLAB = N // NCORES          # 1024 rows per core
MT = SLAB // 128            # 8 m-tiles per core
GROUP = 2048                # columns per PSUM group (4 banks)
NG = N // GROUP             # 4 groups
CHUNK = 512                 # columns per matmul (1 PSUM bank)
NCHUNK = GROUP // CHUNK     # 4 chunks per group
BIG = 5.0e4
BSHIFT = 23.0

BF16 = ml_dtypes.bfloat16
_F = mybir.dt.float32
_B = mybir.dt.bfloat16
_P8 = mybir.dt.float8e4


def _register_hinge_op():
    """Fused custom DVE op for the whole per-element hinge:

        out = max(Src0, C0) + max(Src0, Src1 + C1) - Src0 * C2
        accum_out = sum(out)

    Src0 = D tile (fp32), Src1 = shifted b row (bf16), C0 = a (fp32 [P,1]),
    C1 = BSHIFT, C2 = 2.0.
    """
    name = "CONTRASTIVE_HINGE2_ANT"
    for op in _dve_ops.OPS:
        if op.name == name:
            return op

    def _ref(in0, in1, s0, s1, imm2):
        x = in0.astype(np.float32)
        body = (
            np.maximum(x, s0)
            + np.maximum(x, in1.astype(np.float32) + s1)
            - x * imm2
        ).astype(np.float32)
        return body, body.reshape(body.shape[0], -1).sum(axis=-1, keepdims=True)

    from operator import add as _add

    spec = _DveSpec(
        body=_dve_maxx(Src0, C0) + _dve_maxx(Src0, Src1 + C1) - Src0 * _C2,
        accum=_add,
        reference=_ref,
    )
    # sha pinned at runtime from our own lowering (no source tree to edit)
    shas = {}
    for ver in ("v3", "v4"):
        try:
            s = _DveOpSpec(
                name=name, opcode=0, uops=_dve_lower(spec, ver=ver), rd1_en=True
            )
            shas[ver] = s.sha(ver)
        except Exception:
            pass
    op = _dve_ops.DveOp(name, spec, subdim=False, uops_sha=shas)
    _dve_ops.OPS.append(op)
    _dve_ops._SUB_OPCODE_FOR_NAME[op.name] = (
        _dve_ops._CUSTOM_DVE_ROW_BASE + len(_dve_ops.OPS) - 1
    )
    return op


def build_module():
    """Trace + compile the per-core Bass module (one SPMD NEFF, 8 cores)."""
    nc = bacc.Bacc("TRN2", num_devices=NCORES)

    # fp8 DoubleRow dot operands: k = p + 128*i for tile [p, i, :].
    # dot = im_hi@s_hi + im_lo@s_hi + im_hi@s_lo (lo*lo dropped, ~1e-2 abs).
    imTh = nc.dram_tensor("imTh", [2, 128, SLAB], _P8, kind="ExternalInput")
    imTl = nc.dram_tensor("imTl", [2, 128, SLAB], _P8, kind="ExternalInput")
    sTh = nc.dram_tensor("sTh", [2, 128, N], _P8, kind="ExternalInput")
    sTl = nc.dram_tensor("sTl", [2, 128, N], _P8, kind="ExternalInput")
    fold = nc.dram_tensor("fold", [2, 2, N], _P8, kind="ExternalInput")
    brow = nc.dram_tensor("brow", [N], _B, kind="ExternalInput")
    avec = nc.dram_tensor("avec", [128, MT], _F, kind="ExternalInput")
    imsq = nc.dram_tensor("imsq", [128, MT], _F, kind="ExternalInput")
    ident = nc.dram_tensor("ident", [128, 128], _B, kind="ExternalInput")
    nident = nc.dram_tensor("nident", [128, 128], _B, kind="ExternalInput")
    out = nc.dram_tensor("out", [1, 1], _F, kind="ExternalOutput")

    dbufs = int(os.environ.get("DBUFS", "3"))
    tbufs = int(os.environ.get("TBUFS", "2"))

    with tile.TileContext(nc) as tc:
        with (
            tc.tile_pool(name="singles", bufs=1) as singles,
            tc.tile_pool(name="dtiles", bufs=dbufs) as dpool,
            tc.tile_pool(name="trash", bufs=tbufs) as tpool,
            tc.tile_pool(name="psum", bufs=2, space="PSUM") as ppool,
        ):
            # ---- DMAs ---------------------------------------------------------
            # Matmul hi-operands stream on the sync (HWDGE) queue in first-use
            # order; lo-operands, constants, fold and b go via gpsimd (SWDGE)
            # in parallel so the sync queue dispatch (565ns per DMA) stays
            # short.
            lhs_h = singles.tile([128, 2, SLAB], _P8)
            lhs_l = singles.tile([128, 2, SLAB], _P8)
            rhs_h = singles.tile([128, 2, N], _P8)
            rhs_l = singles.tile([128, 2, N], _P8)
            fold_sb = singles.tile([128, 2, N], _P8)
            b_sb = singles.tile([128, N], _B)
            avec_sb = singles.tile([128, MT], _F)
            imsq_sb = singles.tile([128, MT], _F)
            ident_sb = singles.tile([128, 128], _B)
            nident_sb = singles.tile([128, 128], _B)

            nc.sync.dma_start(out=lhs_h[:, 0, 0:128], in_=imTh.ap()[0, :, 0:128])
            nc.sync.dma_start(out=lhs_h[:, 1, 0:128], in_=imTh.ap()[1, :, 0:128])
            nc.gpsimd.dma_start(out=lhs_l[:, 0, 0:128], in_=imTl.ap()[0, :, 0:128])
            nc.gpsimd.dma_start(out=lhs_l[:, 1, 0:128], in_=imTl.ap()[1, :, 0:128])
            nc.gpsimd.dma_start(out=ident_sb[:], in_=ident.ap())
            nc.gpsimd.dma_start(out=nident_sb[:], in_=nident.ap())
            for bp in (0, 32, 64, 96):
                nc.gpsimd.dma_start(
                    out=fold_sb[bp : bp + 2, :, :], in_=fold.ap()
                )
            nc.gpsimd.dma_start(out=avec_sb[:], in_=avec.ap())
            nc.gpsimd.dma_start(out=imsq_sb[:], in_=imsq.ap())
            for c in range(NCHUNK):
                cc = slice(c * CHUNK, (c + 1) * CHUNK)
                nc.sync.dma_start(out=rhs_h[:, 0, cc], in_=sTh.ap()[0, :, cc])
                nc.sync.dma_start(out=rhs_h[:, 1, cc], in_=sTh.ap()[1, :, cc])
                nc.gpsimd.dma_start(out=rhs_l[:, 0, cc], in_=sTl.ap()[0, :, cc])
                nc.gpsimd.dma_start(out=rhs_l[:, 1, cc], in_=sTl.ap()[1, :, cc])
            nc.gpsimd.dma_start(
                out=b_sb[:, 0:GROUP],
                in_=bass.AP(
                    tensor=brow.ap().tensor, offset=0, ap=[[0, 128], [1, GROUP]]
                ),
            )
            nc.sync.dma_start(
                out=lhs_h[:, 0, 128:SLAB], in_=imTh.ap()[0, :, 128:SLAB]
            )
            nc.sync.dma_start(
                out=lhs_h[:, 1, 128:SLAB], in_=imTh.ap()[1, :, 128:SLAB]
            )
            nc.gpsimd.dma_start(
                out=lhs_l[:, 0, 128:SLAB], in_=imTl.ap()[0, :, 128:SLAB]
            )
            nc.gpsimd.dma_start(
                out=lhs_l[:, 1, 128:SLAB], in_=imTl.ap()[1, :, 128:SLAB]
            )
            # remaining column groups stream in behind
            for g in range(1, NG):
                cols = slice(g * GROUP, (g + 1) * GROUP)
                nc.sync.dma_start(out=rhs_h[:, 0, cols], in_=sTh.ap()[0, :, cols])
                nc.sync.dma_start(out=rhs_h[:, 1, cols], in_=sTh.ap()[1, :, cols])
                nc.gpsimd.dma_start(out=rhs_l[:, 0, cols], in_=sTl.ap()[0, :, cols])
                nc.gpsimd.dma_start(out=rhs_l[:, 1, cols], in_=sTl.ap()[1, :, cols])
                nc.gpsimd.dma_start(
                    out=b_sb[:, cols],
                    in_=bass.AP(
                        tensor=brow.ap().tensor,
                        offset=g * GROUP,
                        ap=[[0, 128], [1, GROUP]],
                    ),
                )

            ones8 = singles.tile([128, 2, 128], _P8)
            nc.vector.memset(ones8[:], 1.0)
            ones_col = singles.tile([128, 1], _F)
            nc.vector.memset(ones_col[:], 1.0)

            acc = singles.tile([128, MT * NG], _F)
            hinge_op = _register_hinge_op()
            fold_pack = os.environ.get("FOLD_PACK", "1") == "1"

            # ---- main loop (group-major: early compute only needs rhs g0) ----
            for g in range(NG):
                for m in range(MT):
                    mm = slice(m * 128, (m + 1) * 128)
                    a_col = avec_sb[:, m : m + 1]
                    q_col = imsq_sb[:, m : m + 1]
                    ps = ppool.tile([128, GROUP], _F, tag="psum")
                    for c in range(NCHUNK):
                        pslice = ps[:, c * CHUNK : (c + 1) * CHUNK]
                        cols = slice(
                            g * GROUP + c * CHUNK, g * GROUP + (c + 1) * CHUNK
                        )
                        nc.tensor.matmul(
                            pslice, lhsT=lhs_h[:, :, mm], rhs=rhs_h[:, :, cols],
                            start=True, stop=False,
                            perf_mode=mybir.MatmulPerfMode.DoubleRow,
                        )
                        nc.tensor.matmul(
                            pslice, lhsT=lhs_l[:, :, mm], rhs=rhs_h[:, :, cols],
                            start=False, stop=False,
                            perf_mode=mybir.MatmulPerfMode.DoubleRow,
                        )
                        nc.tensor.matmul(
                            pslice, lhsT=lhs_h[:, :, mm], rhs=rhs_l[:, :, cols],
                            start=False, stop=False,
                            perf_mode=mybir.MatmulPerfMode.DoubleRow,
                        )
                        if g == 0 and c == (m * 128) // CHUNK:
                            # -BIG onto this m-tile's diagonal block (local
                            # cols [128m, 128m+128)), entirely on PE:
                            # ident.T @ (-BIG*ident) accumulated into the
                            # 128-wide psum slice at the diagonal offset.
                            o = (m * 128) % CHUNK
                            nc.tensor.matmul(
                                ps[:, m * 128 : m * 128 + 128],
                                lhsT=ident_sb[:], rhs=nident_sb[:],
                                start=False, stop=False,
                            )
                    # K=4 fp8 DoubleRow fold matmuls (add -s_sq/2), packed
                    # 4-wide into disjoint PE row groups so they overlap.
                    for c in range(NCHUNK):
                        bp = 32 * c if fold_pack else 0
                        pslice = ps[:, c * CHUNK : (c + 1) * CHUNK]
                        cols = slice(
                            g * GROUP + c * CHUNK, g * GROUP + (c + 1) * CHUNK
                        )
                        nc.tensor.matmul(
                            pslice,
                            lhsT=ones8[bp : bp + 2, :, :],
                            rhs=fold_sb[bp : bp + 2, :, cols],
                            start=False, stop=True,
                            perf_mode=mybir.MatmulPerfMode.DoubleRow,
                            tile_position=(bp, 0),
                        )
                    col = m * NG + g
                    dt = dpool.tile([128, GROUP], _F, tag="dt")
                    nc.scalar.activation(
                        out=dt[:], in_=ps[:],
                        func=mybir.ActivationFunctionType.Sqrt,
                        bias=q_col, scale=-2.0,
                    )
                    t1 = tpool.tile([128, GROUP], _F, tag="t1")
                    nc.vector._custom_dve(
                        hinge_op,
                        out=t1[:],
                        in0=dt[:],
                        in1=b_sb[:, g * GROUP : (g + 1) * GROUP],
                        s0=a_col,
                        s1=BSHIFT,
                        imm2=2.0,
                        accum_out=acc[:, col : col + 1],
                    )

            # ---- final combine ----------------------------------------------
            total_col = singles.tile([128, 1], _F)
            nc.vector.tensor_reduce(
                out=total_col[:], in_=acc[:], axis=mybir.AxisListType.X,
                op=mybir.AluOpType.add,
            )
            fps = ppool.tile([1, 1], _F, tag="psum")
            nc.tensor.matmul(
                fps[:], lhsT=total_col[:], rhs=ones_col[:], start=True, stop=True
            )
            out_sb = singles.tile([1, 1], _F)
            nc.vector.tensor_copy(out=out_sb[:], in_=fps[:])
            nc.sync.dma_start(out=out.ap(), in_=out_sb[:])

    nc.compile()
    return nc


def prepare_inputs(im: np.ndarray, s: np.ndarray):
    """Host-side sharding + dtype conversion. Returns in_maps for 8 cores."""
    im = np.ascontiguousarray(im, dtype=np.float32)
    s = np.ascontiguousarray(s, dtype=np.float32)

    im64 = im.astype(np.float64)
    s64 = s.astype(np.float64)
    diag_true = np.sqrt(((im64 - s64) ** 2).sum(1))          # [N] exact
    b_full = (MARGIN + diag_true).astype(np.float32)         # [N] exact fp32

    _f8 = mybir.dt.np(_P8)
    im_hi = im.astype(_f8)
    im_lo = (im - im_hi.astype(np.float32)).astype(_f8)
    s_hi = s.astype(_f8)
    s_lo = (s - s_hi.astype(np.float32)).astype(_f8)
    im_q64 = im_hi.astype(np.float64) + im_lo.astype(np.float64)
    s_q64 = s_hi.astype(np.float64) + s_lo.astype(np.float64)
    im_sq = (im_q64 ** 2).sum(1).astype(np.float32)                   # [N]
    s_sq = (s_q64 ** 2).sum(1)                                        # [N] f64

    # 4-term fp8 split of -s_sq/2 (residual splitting; |err| < 1e-3)
    foldv = -0.5 * s_sq
    frs, rem = [], foldv.copy()
    for _ in range(4):
        r = rem.astype(np.float32).astype(_f8)
        frs.append(r)
        rem = rem - r.astype(np.float64)

    ident = np.eye(128, dtype=np.float32).astype(BF16)
    nident = (np.eye(128, dtype=np.float32) * np.float32(-BIG)).astype(BF16)

    in_maps = []
    for c in range(NCORES):
        rows = slice(c * SLAB, (c + 1) * SLAB)
        rot = np.roll(np.arange(N), -c * SLAB)
        imTh = np.ascontiguousarray(im_hi[rows].T.reshape(2, 128, SLAB))
        imTl = np.ascontiguousarray(im_lo[rows].T.reshape(2, 128, SLAB))
        sTh = np.ascontiguousarray(s_hi[rot].T.reshape(2, 128, N))
        sTl = np.ascontiguousarray(s_lo[rot].T.reshape(2, 128, N))
        # DoubleRow K layout: k = p + 2*i for tile [p, i, :]
        foldc = np.ascontiguousarray(
            np.stack(
                [
                    np.stack([frs[0][rot], frs[2][rot]]),
                    np.stack([frs[1][rot], frs[3][rot]]),
                ]
            )
        )
        browc = np.ascontiguousarray(
            (b_full[rot] - np.float32(BSHIFT)).astype(BF16)
        )
        avecc = np.ascontiguousarray(b_full[rows].reshape(MT, 128).T)
        imsqc = np.ascontiguousarray(im_sq[rows].reshape(MT, 128).T)
        in_maps.append(
            {
                "imTh": imTh,
                "imTl": imTl,
                "sTh": sTh,
                "sTl": sTl,
                "fold": foldc,
                "brow": browc,
                "avec": avecc,
                "imsq": imsqc,
                "ident": ident,
                "nident": nident,
            }
        )
    return in_maps


_NC_CACHE = None


def get_module():
    global _NC_CACHE
    if _NC_CACHE is None:
        _NC_CACHE = build_module()
    return _NC_CACHE


def kernel(im: np.ndarray, s: np.ndarray) -> np.ndarray:
    nc = get_module()
    in_maps = prepare_inputs(im, s)
    res = bass_utils.run_bass_kernel_spmd(
        nc, in_maps, core_ids=list(range(NCORES))
    )
    total = 0.0
    for c in range(NCORES):
        total += float(res.results[c]["out"][0, 0])
    return np.array(np.float64(total) / (N * N), dtype=np.float32)



# revision 12
# speedup vs baseline: 1.0256x; 1.0256x over previous
# Contrastive loss (L2-distance scores, margin hinge, mean reduction) on 8
# Trainium2 NeuronCores.
#
# total = mean(cost_s) + mean(cost_im) over the [N, N] score matrix
#   scores[i, j] = -||im_i - s_j||;  D_ij = ||im_i - s_j||
#   cost_s  = relu(margin + scores - diag_row)   (diag zeroed)
#   cost_im = relu(margin + scores - diag_col)   (diag zeroed)
#
# Device identity:  relu(a - d) = max(a, d) - d, so with a_i = b_i =
# margin + D_ii:
#   sum(cost_s) + sum(cost_im) = sum_ij [max(a_i, D_ij) + max(b_j, D_ij) - 2 D_ij]
#
# Sharding: rows of the score matrix across 8 cores ([1024, 8192] slab each).
# s (columns) is replicated but column-ROTATED per core so every core's
# diagonal block lands at local columns [0, 1024) -> one static SPMD program.
# The diagonal is zeroed exactly by accumulating -BIG into the PSUM diagonal
# (rank-128 identity matmul): D_ii becomes ~sqrt(2*BIG) >> a, b, so
# max(a, D_ii) + max(b, D_ii) - 2 D_ii = 0 exactly.
#
# Per (m-tile of 128 rows, group of 2048 cols = 4 PSUM banks):
#   PE  : per 512-chunk: 2x K=128 bf16 matmuls (dot) + [diag group only]
#         ident x (-BIG*ident) boost + 1x K=4 fp8 DoubleRow matmul adding a
#         4-term fp8 split of -||s_j||^2/2  ->  PSUM q = dot - s_sq/2 (- BIG
#         on diag)
#   ACT : D = sqrt(-2*q + ||im_i||^2)  [PSUM -> SBUF fp32]
#   DVE : one fused custom op:  out = max(D, a_i) + max(D, b_j + BSHIFT)
#         - 2*D, accum_out[p] = sum(out)   (b is stored bf16 SHIFTED by
#         -BSHIFT so its bf16 ulp is ~16x smaller; a stays fp32)
# Final: reduce the 32 accumulator columns, partition-sum via a ones-matmul,
# DMA the per-core scalar out. Host: sum 8 partials, divide by N^2.
#
# Numerics vs the fp32 reference: bf16 point quantization dominates
# (~4e-5 relative); measured ~9e-5 end to end.

import os

import numpy as np
import ml_dtypes

import concourse.bass as bass
import concourse.tile as tile
from concourse import bacc, mybir
from concourse import bass_utils
from concourse import dve_ops as _dve_ops
from concourse.dve_spec import (
    Spec as _DveSpec,
    Src0,
    Src1,
    C0,
    C1,
    C2 as _C2,
    lower as _dve_lower,
    maxx as _dve_maxx,
)
from concourse.dve_uop import DveOpSpec as _DveOpSpec

N = 8192
D = 256
MARGIN = 0.2
NCORES = 8
SLAB = N // NCORES          # 1024 rows per core
MT = SLAB // 128            # 8 m-tiles per core
GROUP = 2048                # columns per PSUM group (4 banks)
NG = N // GROUP             # 4 groups
CHUNK = 512                 # columns per matmul (1 PSUM bank)
NCHUNK = GROUP // CHUNK     # 4 chunks per group
BIG = 5.0e4
BSHIFT = 23.0

BF16 = ml_dtypes.bfloat16
_F = mybir.dt.float32
_B = mybir.dt.bfloat16
_P8 = mybir.dt.float8e4


def _register_hinge_op():
    """Fused custom DVE op for the whole per-element hinge:

        out = max(Src0, C0) + max(Src0, Src1 + C1) - Src0 * C2
        accum_out = sum(out)

    Src0 = D tile (fp32), Src1 = shifted b row (bf16), C0 = a (fp32 [P,1]),
    C1 = BSHIFT, C2 = 2.0.
    """
    name = "CONTRASTIVE_HINGE2_ANT"
    for op in _dve_ops.OPS:
        if op.name == name:
            return op

    def _ref(in0, in1, s0, s1, imm2):
        x = in0.astype(np.float32)
        body = (
            np.maximum(x, s0)
            + np.maximum(x, in1.astype(np.float32) + s1)
            - x * imm2
        ).astype(np.float32)
        return body, body.reshape(body.shape[0], -1).sum(axis=-1, keepdims=True)

    from operator import add as _add

    spec = _DveSpec(
        body=_dve_maxx(Src0, C0) + _dve_maxx(Src0, Src1 + C1) - Src0 * _C2,
        accum=_add,
        reference=_ref,
    )
    # sha pinned at runtime from our own lowering (no source tree to edit)
    shas = {}
    for ver in ("v3", "v4"):
        try:
            s = _DveOpSpec(
                name=name, opcode=0, uops=_dve_lower(spec, ver=ver), rd1_en=True
            )
            shas[ver] = s.sha(ver)
        except Exception:
            pass
    op = _dve_ops.DveOp(name, spec, subdim=False, uops_sha=shas)
    _dve_ops.OPS.append(op)
    _dve_ops._SUB_OPCODE_FOR_NAME[op.name] = (
        _dve_ops._CUSTOM_DVE_ROW_BASE + len(_dve_ops.OPS) - 1
    )
    return op


def build_module():
    """Trace + compile the per-core Bass module (one SPMD NEFF, 8 cores)."""
    nc = bacc.Bacc("TRN2", num_devices=NCORES)

    # fp8 DoubleRow dot operands: k = p + 128*i for tile [p, i, :].
    # dot = im_hi@s_hi + im_lo@s_hi + im_hi@s_lo (lo*lo dropped, ~1e-2 abs).
    imTh = nc.dram_tensor("imTh", [2, 128, SLAB], _P8, kind="ExternalInput")
    imTl = nc.dram_tensor("imTl", [2, 128, SLAB], _P8, kind="ExternalInput")
    sTh = nc.dram_tensor("sTh", [2, 128, N], _P8, kind="ExternalInput")
    sTl = nc.dram_tensor("sTl", [2, 128, N], _P8, kind="ExternalInput")
    fold = nc.dram_tensor("fold", [2, 2, N], _P8, kind="ExternalInput")
    brow = nc.dram_tensor("brow", [N], _B, kind="ExternalInput")
    avec = nc.dram_tensor("avec", [128, MT], _F, kind="ExternalInput")
    imsq = nc.dram_tensor("imsq", [128, MT], _F, kind="ExternalInput")
    ident = nc.dram_tensor("ident", [128, 128], _B, kind="ExternalInput")
    nident = nc.dram_tensor("nident", [128, 128], _B, kind="ExternalInput")
    out = nc.dram_tensor("out", [1, 1], _F, kind="ExternalOutput")

    dbufs = int(os.environ.get("DBUFS", "3"))
    tbufs = int(os.environ.get("TBUFS", "2"))
    dot_terms = int(os.environ.get("DOT_TERMS", "2"))

    with tile.TileContext(nc) as tc:
        with (
            tc.tile_pool(name="singles", bufs=1) as singles,
            tc.tile_pool(name="dtiles", bufs=dbufs) as dpool,
            tc.tile_pool(name="trash", bufs=tbufs) as tpool,
            tc.tile_pool(name="psum", bufs=2, space="PSUM") as ppool,
        ):
            # ---- DMAs ---------------------------------------------------------
            # Matmul hi-operands stream on the sync (HWDGE) queue in first-use
            # order; lo-operands, constants, fold and b go via gpsimd (SWDGE)
            # in parallel so the sync queue dispatch (565ns per DMA) stays
            # short.
            lhs_h = singles.tile([128, 2, SLAB], _P8)
            lhs_l = singles.tile([128, 2, SLAB], _P8)
            rhs_h = singles.tile([128, 2, N], _P8)
            rhs_l = singles.tile([128, 2, N], _P8)
            fold_sb = singles.tile([128, 2, N], _P8)
            b_sb = singles.tile([128, N], _B)
            avec_sb = singles.tile([128, MT], _F)
            imsq_sb = singles.tile([128, MT], _F)
            ident_sb = singles.tile([128, 128], _B)
            nident_sb = singles.tile([128, 128], _B)

            nc.sync.dma_start(out=lhs_h[:, 0, 0:128], in_=imTh.ap()[0, :, 0:128])
            nc.sync.dma_start(out=lhs_h[:, 1, 0:128], in_=imTh.ap()[1, :, 0:128])
            if dot_terms >= 2:
                nc.gpsimd.dma_start(out=lhs_l[:, 0, 0:128], in_=imTl.ap()[0, :, 0:128])
                nc.gpsimd.dma_start(out=lhs_l[:, 1, 0:128], in_=imTl.ap()[1, :, 0:128])
            nc.gpsimd.dma_start(out=ident_sb[:], in_=ident.ap())
            nc.gpsimd.dma_start(out=nident_sb[:], in_=nident.ap())
            for bp in (0, 32, 64, 96):
                nc.gpsimd.dma_start(
                    out=fold_sb[bp : bp + 2, :, :], in_=fold.ap()
                )
            nc.gpsimd.dma_start(out=avec_sb[:], in_=avec.ap())
            nc.gpsimd.dma_start(out=imsq_sb[:], in_=imsq.ap())
            for c in range(NCHUNK):
                cc = slice(c * CHUNK, (c + 1) * CHUNK)
                nc.sync.dma_start(out=rhs_h[:, 0, cc], in_=sTh.ap()[0, :, cc])
                nc.sync.dma_start(out=rhs_h[:, 1, cc], in_=sTh.ap()[1, :, cc])
                if dot_terms >= 3:
                    nc.gpsimd.dma_start(out=rhs_l[:, 0, cc], in_=sTl.ap()[0, :, cc])
                    nc.gpsimd.dma_start(out=rhs_l[:, 1, cc], in_=sTl.ap()[1, :, cc])
            nc.gpsimd.dma_start(
                out=b_sb[:, 0:GROUP],
                in_=bass.AP(
                    tensor=brow.ap().tensor, offset=0, ap=[[0, 128], [1, GROUP]]
                ),
            )
            nc.sync.dma_start(
                out=lhs_h[:, 0, 128:SLAB], in_=imTh.ap()[0, :, 128:SLAB]
            )
            nc.sync.dma_start(
                out=lhs_h[:, 1, 128:SLAB], in_=imTh.ap()[1, :, 128:SLAB]
            )
            if dot_terms >= 2:
                nc.gpsimd.dma_start(
                    out=lhs_l[:, 0, 128:SLAB], in_=imTl.ap()[0, :, 128:SLAB]
                )
                nc.gpsimd.dma_start(
                    out=lhs_l[:, 1, 128:SLAB], in_=imTl.ap()[1, :, 128:SLAB]
                )
            # remaining column groups stream in behind
            for g in range(1, NG):
                cols = slice(g * GROUP, (g + 1) * GROUP)
                nc.sync.dma_start(out=rhs_h[:, 0, cols], in_=sTh.ap()[0, :, cols])
                nc.sync.dma_start(out=rhs_h[:, 1, cols], in_=sTh.ap()[1, :, cols])
                if dot_terms >= 3:
                    nc.gpsimd.dma_start(out=rhs_l[:, 0, cols], in_=sTl.ap()[0, :, cols])
                    nc.gpsimd.dma_start(out=rhs_l[:, 1, cols], in_=sTl.ap()[1, :, cols])
                nc.gpsimd.dma_start(
                    out=b_sb[:, cols],
                    in_=bass.AP(
                        tensor=brow.ap().tensor,
                        offset=g * GROUP,
                        ap=[[0, 128], [1, GROUP]],
                    ),
                )

            ones8 = singles.tile([128, 2, 128], _P8)
            nc.vector.memset(ones8[:], 1.0)
            ones_col = singles.tile([128, 1], _F)
            nc.vector.memset(ones_col[:], 1.0)

            acc = singles.tile([128, MT * NG], _F)
            hinge_op = _register_hinge_op()
            fold_pack = os.environ.get("FOLD_PACK", "1") == "1"

            # ---- main loop (group-major: early compute only needs rhs g0) ----
            for g in range(NG):
                for m in range(MT):
                    mm = slice(m * 128, (m + 1) * 128)
                    a_col = avec_sb[:, m : m + 1]
                    q_col = imsq_sb[:, m : m + 1]
                    ps = ppool.tile([128, GROUP], _F, tag="psum")
                    for c in range(NCHUNK):
                        pslice = ps[:, c * CHUNK : (c + 1) * CHUNK]
                        cols = slice(
                            g * GROUP + c * CHUNK, g * GROUP + (c + 1) * CHUNK
                        )
                        nc.tensor.matmul(
                            pslice, lhsT=lhs_h[:, :, mm], rhs=rhs_h[:, :, cols],
                            start=True, stop=False,
                            perf_mode=mybir.MatmulPerfMode.DoubleRow,
                        )
                        if dot_terms >= 2:
                            nc.tensor.matmul(
                                pslice, lhsT=lhs_l[:, :, mm], rhs=rhs_h[:, :, cols],
                                start=False, stop=False,
                                perf_mode=mybir.MatmulPerfMode.DoubleRow,
                            )
                        if dot_terms >= 3:
                            nc.tensor.matmul(
                                pslice, lhsT=lhs_h[:, :, mm], rhs=rhs_l[:, :, cols],
                                start=False, stop=False,
                                perf_mode=mybir.MatmulPerfMode.DoubleRow,
                            )
                        if g == 0 and c == (m * 128) // CHUNK:
                            # -BIG onto this m-tile's diagonal block (local
                            # cols [128m, 128m+128)), entirely on PE:
                            # ident.T @ (-BIG*ident) accumulated into the
                            # 128-wide psum slice at the diagonal offset.
                            o = (m * 128) % CHUNK
                            nc.tensor.matmul(
                                ps[:, m * 128 : m * 128 + 128],
                                lhsT=ident_sb[:], rhs=nident_sb[:],
                                start=False, stop=False,
                            )
                    # K=4 fp8 DoubleRow fold matmuls (add -s_sq/2), packed
                    # 4-wide into disjoint PE row groups so they overlap.
                    for c in range(NCHUNK):
                        bp = 32 * c if fold_pack else 0
                        pslice = ps[:, c * CHUNK : (c + 1) * CHUNK]
                        cols = slice(
                            g * GROUP + c * CHUNK, g * GROUP + (c + 1) * CHUNK
                        )
                        nc.tensor.matmul(
                            pslice,
                            lhsT=ones8[bp : bp + 2, :, :],
                            rhs=fold_sb[bp : bp + 2, :, cols],
                            start=False, stop=True,
                            perf_mode=mybir.MatmulPerfMode.DoubleRow,
                            tile_position=(bp, 0),
                        )
                    col = m * NG + g
                    dt = dpool.tile([128, GROUP], _F, tag="dt")
                    nc.scalar.activation(
                        out=dt[:], in_=ps[:],
                        func=mybir.ActivationFunctionType.Sqrt,
                        bias=q_col, scale=-2.0,
                    )
                    t1 = tpool.tile([128, GROUP], _F, tag="t1")
                    nc.vector._custom_dve(
                        hinge_op,
                        out=t1[:],
                        in0=dt[:],
                        in1=b_sb[:, g * GROUP : (g + 1) * GROUP],
                        s0=a_col,
                        s1=BSHIFT,
                        imm2=2.0,
                        accum_out=acc[:, col : col + 1],
                    )

            # ---- final combine ----------------------------------------------
            total_col = singles.tile([128, 1], _F)
            nc.vector.tensor_reduce(
                out=total_col[:], in_=acc[:], axis=mybir.AxisListType.X,
                op=mybir.AluOpType.add,
            )
            fps = ppool.tile([1, 1], _F, tag="psum")
            nc.tensor.matmul(
                fps[:], lhsT=total_col[:], rhs=ones_col[:], start=True, stop=True
            )
            out_sb = singles.tile([1, 1], _F)
            nc.vector.tensor_copy(out=out_sb[:], in_=fps[:])
            nc.sync.dma_start(out=out.ap(), in_=out_sb[:])

    nc.compile()
    return nc


def prepare_inputs(im: np.ndarray, s: np.ndarray):
    """Host-side sharding + dtype conversion. Returns in_maps for 8 cores."""
    im = np.ascontiguousarray(im, dtype=np.float32)
    s = np.ascontiguousarray(s, dtype=np.float32)

    im64 = im.astype(np.float64)
    s64 = s.astype(np.float64)
    diag_true = np.sqrt(((im64 - s64) ** 2).sum(1))          # [N] exact
    b_full = (MARGIN + diag_true).astype(np.float32)         # [N] exact fp32

    _f8 = mybir.dt.np(_P8)
    im_hi = im.astype(_f8)
    im_lo = (im - im_hi.astype(np.float32)).astype(_f8)
    s_hi = s.astype(_f8)
    s_lo = (s - s_hi.astype(np.float32)).astype(_f8)
    dot_terms = int(os.environ.get("DOT_TERMS", "2"))
    im_q64 = im_hi.astype(np.float64) + (
        im_lo.astype(np.float64) if dot_terms >= 2 else 0.0
    )
    s_q64 = s_hi.astype(np.float64) + (
        s_lo.astype(np.float64) if dot_terms >= 3 else 0.0
    )
    im_sq = (im_q64 ** 2).sum(1).astype(np.float32)                   # [N]
    s_sq = (s_q64 ** 2).sum(1)                                        # [N] f64

    # 4-term fp8 split of -s_sq/2 (residual splitting; |err| < 1e-3)
    foldv = -0.5 * s_sq
    frs, rem = [], foldv.copy()
    for _ in range(4):
        r = rem.astype(np.float32).astype(_f8)
        frs.append(r)
        rem = rem - r.astype(np.float64)

    ident = np.eye(128, dtype=np.float32).astype(BF16)
    nident = (np.eye(128, dtype=np.float32) * np.float32(-BIG)).astype(BF16)

    in_maps = []
    for c in range(NCORES):
        rows = slice(c * SLAB, (c + 1) * SLAB)
        rot = np.roll(np.arange(N), -c * SLAB)
        imTh = np.ascontiguousarray(im_hi[rows].T.reshape(2, 128, SLAB))
        imTl = np.ascontiguousarray(im_lo[rows].T.reshape(2, 128, SLAB))
        sTh = np.ascontiguousarray(s_hi[rot].T.reshape(2, 128, N))
        sTl = np.ascontiguousarray(s_lo[rot].T.reshape(2, 128, N))
        # DoubleRow K layout: k = p + 2*i for tile [p, i, :]
        foldc = np.ascontiguousarray(
            np.stack(
                [
                    np.stack([frs[0][rot], frs[2][rot]]),
                    np.stack([frs[1][rot], frs[3][rot]]),
                ]
            )
        )
        browc = np.ascontiguousarray(
            (b_full[rot] - np.float32(BSHIFT)).astype(BF16)
        )
        avecc = np.ascontiguousarray(b_full[rows].reshape(MT, 128).T)
        imsqc = np.ascontiguousarray(im_sq[rows].reshape(MT, 128).T)
        in_maps.append(
            {
                "imTh": imTh,
                "imTl": imTl,
                "sTh": sTh,
                "sTl": sTl,
                "fold": foldc,
                "brow": browc,
                "avec": avecc,
                "imsq": imsqc,
                "ident": ident,
                "nident": nident,
            }
        )
    return in_maps


_NC_CACHE = None


def get_module():
    global _NC_CACHE
    if _NC_CACHE is None:
        _NC_CACHE = build_module()
    return _NC_CACHE


def kernel(im: np.ndarray, s: np.ndarray) -> np.ndarray:
    nc = get_module()
    in_maps = prepare_inputs(im, s)
    res = bass_utils.run_bass_kernel_spmd(
        nc, in_maps, core_ids=list(range(NCORES))
    )
    total = 0.0
    for c in range(NCORES):
        total += float(res.results[c]["out"][0, 0])
    return np.array(np.float64(total) / (N * N), dtype=np.float32)



# revision 13
# speedup vs baseline: 1.1370x; 1.1086x over previous
# Contrastive loss (L2-distance scores, margin hinge, mean reduction) on 8
# Trainium2 NeuronCores.
#
# total = mean(cost_s) + mean(cost_im) over the [N, N] score matrix
#   scores[i, j] = -||im_i - s_j||;  D_ij = ||im_i - s_j||
#   cost_s  = relu(margin + scores - diag_row)   (diag zeroed)
#   cost_im = relu(margin + scores - diag_col)   (diag zeroed)
#
# Device identity:  relu(a - d) = max(a, d) - d, so with a_i = b_i =
# margin + D_ii:
#   sum(cost_s) + sum(cost_im) = sum_ij [max(a_i, D_ij) + max(b_j, D_ij) - 2 D_ij]
#
# Sharding: rows of the score matrix across 8 cores ([1024, 8192] slab each).
# s (columns) is replicated but column-ROTATED per core so every core's
# diagonal block lands at local columns [0, 1024) -> one static SPMD program.
# The diagonal is zeroed exactly by accumulating -BIG into the PSUM diagonal
# (rank-128 identity matmul): D_ii becomes ~sqrt(2*BIG) >> a, b, so
# max(a, D_ii) + max(b, D_ii) - 2 D_ii = 0 exactly.
#
# Per (m-tile of 128 rows, group of 2048 cols = 4 PSUM banks):
#   PE  : per 512-chunk: 2x K=128 bf16 matmuls (dot) + [diag group only]
#         ident x (-BIG*ident) boost + 1x K=4 fp8 DoubleRow matmul adding a
#         4-term fp8 split of -||s_j||^2/2  ->  PSUM q = dot - s_sq/2 (- BIG
#         on diag)
#   ACT : D = sqrt(-2*q + ||im_i||^2)  [PSUM -> SBUF fp32]
#   DVE : one fused custom op:  out = max(D, a_i) + max(D, b_j + BSHIFT)
#         - 2*D, accum_out[p] = sum(out)   (b is stored bf16 SHIFTED by
#         -BSHIFT so its bf16 ulp is ~16x smaller; a stays fp32)
# Final: reduce the 32 accumulator columns, partition-sum via a ones-matmul,
# DMA the per-core scalar out. Host: sum 8 partials, divide by N^2.
#
# Numerics vs the fp32 reference: bf16 point quantization dominates
# (~4e-5 relative); measured ~9e-5 end to end.

import os

import numpy as np
import ml_dtypes

import concourse.bass as bass
import concourse.tile as tile
from concourse import bacc, mybir
from concourse import bass_utils
from concourse import dve_ops as _dve_ops
from concourse.dve_spec import (
    Spec as _DveSpec,
    Src0,
    Src1,
    C0,
    C1,
    C2 as _C2,
    lower as _dve_lower,
    maxx as _dve_maxx,
)
from concourse.dve_uop import DveOpSpec as _DveOpSpec

N = 8192
D = 256
MARGIN = 0.2
NCORES = 8
SLAB = N // NCORES          # 1024 rows per core
MT = SLAB // 128            # 8 m-tiles per core
GROUP = 2048                # columns per PSUM group (4 banks)
NG = N // GROUP             # 4 groups
CHUNK = 512                 # columns per matmul (1 PSUM bank)
NCHUNK = GROUP // CHUNK     # 4 chunks per group
BIG = 5.0e4
BSHIFT = 23.0

BF16 = ml_dtypes.bfloat16
_F = mybir.dt.float32
_B = mybir.dt.bfloat16
_P8 = mybir.dt.float8e4


def _register_hinge_op():
    """Fused custom DVE op for the whole per-element hinge:

        out = max(Src0, C0) + max(Src0, Src1 + C1) - Src0 * C2
        accum_out = sum(out)

    Src0 = D tile (fp32), Src1 = shifted b row (bf16), C0 = a (fp32 [P,1]),
    C1 = BSHIFT, C2 = 2.0.
    """
    name = "CONTRASTIVE_HINGE2_ANT"
    for op in _dve_ops.OPS:
        if op.name == name:
            return op

    def _ref(in0, in1, s0, s1, imm2):
        x = in0.astype(np.float32)
        body = (
            np.maximum(x, s0)
            + np.maximum(x, in1.astype(np.float32) + s1)
            - x * imm2
        ).astype(np.float32)
        return body, body.reshape(body.shape[0], -1).sum(axis=-1, keepdims=True)

    from operator import add as _add

    spec = _DveSpec(
        body=_dve_maxx(Src0, C0) + _dve_maxx(Src0, Src1 + C1) - Src0 * _C2,
        accum=_add,
        reference=_ref,
    )
    # sha pinned at runtime from our own lowering (no source tree to edit)
    shas = {}
    for ver in ("v3", "v4"):
        try:
            s = _DveOpSpec(
                name=name, opcode=0, uops=_dve_lower(spec, ver=ver), rd1_en=True
            )
            shas[ver] = s.sha(ver)
        except Exception:
            pass
    op = _dve_ops.DveOp(name, spec, subdim=False, uops_sha=shas)
    _dve_ops.OPS.append(op)
    _dve_ops._SUB_OPCODE_FOR_NAME[op.name] = (
        _dve_ops._CUSTOM_DVE_ROW_BASE + len(_dve_ops.OPS) - 1
    )
    return op


def build_module():
    """Trace + compile the per-core Bass module (one SPMD NEFF, 8 cores)."""
    nc = bacc.Bacc("TRN2", num_devices=NCORES)

    # fp8 DoubleRow dot operands: k = p + 128*i for tile [p, i, :].
    # dot = im_hi@s_hi + im_lo@s_hi + im_hi@s_lo (lo*lo dropped, ~1e-2 abs).
    imTh = nc.dram_tensor("imTh", [2, 128, SLAB], _P8, kind="ExternalInput")
    imTl = nc.dram_tensor("imTl", [2, 128, SLAB], _P8, kind="ExternalInput")
    sTh = nc.dram_tensor("sTh", [2, 128, N], _P8, kind="ExternalInput")
    sTl = nc.dram_tensor("sTl", [2, 128, N], _P8, kind="ExternalInput")
    fold = nc.dram_tensor("fold", [2, 2, N], _P8, kind="ExternalInput")
    brow = nc.dram_tensor("brow", [N], _B, kind="ExternalInput")
    avec = nc.dram_tensor("avec", [128, MT], _F, kind="ExternalInput")
    imsq = nc.dram_tensor("imsq", [128, MT], _F, kind="ExternalInput")
    ident = nc.dram_tensor("ident", [128, 128], _B, kind="ExternalInput")
    nident = nc.dram_tensor("nident", [128, 128], _B, kind="ExternalInput")
    out = nc.dram_tensor("out", [1, 1], _F, kind="ExternalOutput")

    dbufs = int(os.environ.get("DBUFS", "3"))
    tbufs = int(os.environ.get("TBUFS", "2"))
    dot_terms = int(os.environ.get("DOT_TERMS", "2"))

    with tile.TileContext(nc) as tc:
        with (
            tc.tile_pool(name="singles", bufs=1) as singles,
            tc.tile_pool(name="dtiles", bufs=dbufs) as dpool,
            tc.tile_pool(name="trash", bufs=tbufs) as tpool,
            tc.tile_pool(name="psum", bufs=2, space="PSUM") as ppool,
        ):
            # ---- DMAs ---------------------------------------------------------
            # Matmul hi-operands stream on the sync (HWDGE) queue in first-use
            # order; lo-operands, constants, fold and b go via gpsimd (SWDGE)
            # in parallel so the sync queue dispatch (565ns per DMA) stays
            # short.
            lhs_h = singles.tile([128, 2, SLAB], _P8)
            lhs_l = singles.tile([128, 2, SLAB], _P8)
            rhs_h = singles.tile([128, 2, N], _P8)
            rhs_l = singles.tile([128, 2, N], _P8)
            fold_sb = singles.tile([128, 2, N], _P8)
            b_sb = singles.tile([128, N], _B)
            avec_sb = singles.tile([128, MT], _F)
            imsq_sb = singles.tile([128, MT], _F)
            ident_sb = singles.tile([128, 128], _B)
            nident_sb = singles.tile([128, 128], _B)

            nc.sync.dma_start(out=lhs_h[:, 0, 0:128], in_=imTh.ap()[0, :, 0:128])
            nc.sync.dma_start(out=lhs_h[:, 1, 0:128], in_=imTh.ap()[1, :, 0:128])
            if dot_terms >= 2:
                nc.gpsimd.dma_start(out=lhs_l[:, 0, 0:128], in_=imTl.ap()[0, :, 0:128])
                nc.gpsimd.dma_start(out=lhs_l[:, 1, 0:128], in_=imTl.ap()[1, :, 0:128])
            nc.gpsimd.dma_start(out=ident_sb[:], in_=ident.ap())
            nc.gpsimd.dma_start(out=nident_sb[:], in_=nident.ap())
            for bp in (0, 32, 64, 96):
                nc.gpsimd.dma_start(
                    out=fold_sb[bp : bp + 2, :, :], in_=fold.ap()
                )
            nc.gpsimd.dma_start(out=avec_sb[:], in_=avec.ap())
            nc.gpsimd.dma_start(out=imsq_sb[:], in_=imsq.ap())
            for c in range(NCHUNK):
                cc = slice(c * CHUNK, (c + 1) * CHUNK)
                nc.sync.dma_start(out=rhs_h[:, 0, cc], in_=sTh.ap()[0, :, cc])
                nc.sync.dma_start(out=rhs_h[:, 1, cc], in_=sTh.ap()[1, :, cc])
                if dot_terms >= 3:
                    nc.gpsimd.dma_start(out=rhs_l[:, 0, cc], in_=sTl.ap()[0, :, cc])
                    nc.gpsimd.dma_start(out=rhs_l[:, 1, cc], in_=sTl.ap()[1, :, cc])
            nc.gpsimd.dma_start(
                out=b_sb[:, 0:GROUP],
                in_=bass.AP(
                    tensor=brow.ap().tensor, offset=0, ap=[[0, 128], [1, GROUP]]
                ),
            )
            nc.sync.dma_start(
                out=lhs_h[:, 0, 128:SLAB], in_=imTh.ap()[0, :, 128:SLAB]
            )
            nc.sync.dma_start(
                out=lhs_h[:, 1, 128:SLAB], in_=imTh.ap()[1, :, 128:SLAB]
            )
            if dot_terms >= 2:
                nc.gpsimd.dma_start(
                    out=lhs_l[:, 0, 128:SLAB], in_=imTl.ap()[0, :, 128:SLAB]
                )
                nc.gpsimd.dma_start(
                    out=lhs_l[:, 1, 128:SLAB], in_=imTl.ap()[1, :, 128:SLAB]
                )
            # remaining column groups stream in behind
            for g in range(1, NG):
                cols = slice(g * GROUP, (g + 1) * GROUP)
                nc.sync.dma_start(out=rhs_h[:, 0, cols], in_=sTh.ap()[0, :, cols])
                nc.sync.dma_start(out=rhs_h[:, 1, cols], in_=sTh.ap()[1, :, cols])
                if dot_terms >= 3:
                    nc.gpsimd.dma_start(out=rhs_l[:, 0, cols], in_=sTl.ap()[0, :, cols])
                    nc.gpsimd.dma_start(out=rhs_l[:, 1, cols], in_=sTl.ap()[1, :, cols])
                nc.gpsimd.dma_start(
                    out=b_sb[:, cols],
                    in_=bass.AP(
                        tensor=brow.ap().tensor,
                        offset=g * GROUP,
                        ap=[[0, 128], [1, GROUP]],
                    ),
                )

            ones8 = singles.tile([128, 2, 128], _P8)
            nc.vector.memset(ones8[:], 1.0)
            ones_col = singles.tile([128, 1], _F)
            nc.vector.memset(ones_col[:], 1.0)

            acc = singles.tile([128, MT * NG], _F)
            hinge_op = _register_hinge_op()
            fold_pack = os.environ.get("FOLD_PACK", "1") == "1"

            # ---- main loop (group-major: early compute only needs rhs g0) ----
            for g in range(NG):
                for m in range(MT):
                    mm = slice(m * 128, (m + 1) * 128)
                    a_col = avec_sb[:, m : m + 1]
                    q_col = imsq_sb[:, m : m + 1]
                    ps = ppool.tile([128, GROUP], _F, tag="psum")
                    for c in range(NCHUNK):
                        pslice = ps[:, c * CHUNK : (c + 1) * CHUNK]
                        cols = slice(
                            g * GROUP + c * CHUNK, g * GROUP + (c + 1) * CHUNK
                        )
                        nc.tensor.matmul(
                            pslice, lhsT=lhs_h[:, :, mm], rhs=rhs_h[:, :, cols],
                            start=True, stop=False,
                            perf_mode=mybir.MatmulPerfMode.DoubleRow,
                        )
                        if dot_terms >= 2:
                            nc.tensor.matmul(
                                pslice, lhsT=lhs_l[:, :, mm], rhs=rhs_h[:, :, cols],
                                start=False, stop=False,
                                perf_mode=mybir.MatmulPerfMode.DoubleRow,
                            )
                        if dot_terms >= 3:
                            nc.tensor.matmul(
                                pslice, lhsT=lhs_h[:, :, mm], rhs=rhs_l[:, :, cols],
                                start=False, stop=False,
                                perf_mode=mybir.MatmulPerfMode.DoubleRow,
                            )
                        if dot_terms >= 4:
                            nc.tensor.matmul(
                                pslice, lhsT=lhs_l[:, :, mm], rhs=rhs_l[:, :, cols],
                                start=False, stop=False,
                                perf_mode=mybir.MatmulPerfMode.DoubleRow,
                            )
                        if g == 0 and c == (m * 128) // CHUNK:
                            # -BIG onto this m-tile's diagonal block (local
                            # cols [128m, 128m+128)), entirely on PE:
                            # ident.T @ (-BIG*ident) accumulated into the
                            # 128-wide psum slice at the diagonal offset.
                            o = (m * 128) % CHUNK
                            nc.tensor.matmul(
                                ps[:, m * 128 : m * 128 + 128],
                                lhsT=ident_sb[:], rhs=nident_sb[:],
                                start=False, stop=False,
                            )
                    # K=4 fp8 DoubleRow fold matmuls (add -s_sq/2), packed
                    # 4-wide into disjoint PE row groups so they overlap.
                    for c in range(NCHUNK):
                        bp = 32 * c if fold_pack else 0
                        pslice = ps[:, c * CHUNK : (c + 1) * CHUNK]
                        cols = slice(
                            g * GROUP + c * CHUNK, g * GROUP + (c + 1) * CHUNK
                        )
                        nc.tensor.matmul(
                            pslice,
                            lhsT=ones8[bp : bp + 2, :, :],
                            rhs=fold_sb[bp : bp + 2, :, cols],
                            start=False, stop=True,
                            perf_mode=mybir.MatmulPerfMode.DoubleRow,
                            tile_position=(bp, 0),
                        )
                    col = m * NG + g
                    dt = dpool.tile([128, GROUP], _F, tag="dt")
                    nc.scalar.activation(
                        out=dt[:], in_=ps[:],
                        func=mybir.ActivationFunctionType.Sqrt,
                        bias=q_col, scale=-2.0,
                    )
                    t1 = tpool.tile([128, GROUP], _F, tag="t1")
                    nc.vector._custom_dve(
                        hinge_op,
                        out=t1[:],
                        in0=dt[:],
                        in1=b_sb[:, g * GROUP : (g + 1) * GROUP],
                        s0=a_col,
                        s1=BSHIFT,
                        imm2=2.0,
                        accum_out=acc[:, col : col + 1],
                    )

            # ---- final combine ----------------------------------------------
            total_col = singles.tile([128, 1], _F)
            nc.vector.tensor_reduce(
                out=total_col[:], in_=acc[:], axis=mybir.AxisListType.X,
                op=mybir.AluOpType.add,
            )
            fps = ppool.tile([1, 1], _F, tag="psum")
            nc.tensor.matmul(
                fps[:], lhsT=total_col[:], rhs=ones_col[:], start=True, stop=True
            )
            out_sb = singles.tile([1, 1], _F)
            nc.vector.tensor_copy(out=out_sb[:], in_=fps[:])
            nc.sync.dma_start(out=out.ap(), in_=out_sb[:])

    nc.compile()
    return nc


def prepare_inputs(im: np.ndarray, s: np.ndarray):
    """Host-side sharding + dtype conversion. Returns in_maps for 8 cores."""
    im = np.ascontiguousarray(im, dtype=np.float32)
    s = np.ascontiguousarray(s, dtype=np.float32)

    im64 = im.astype(np.float64)
    s64 = s.astype(np.float64)
    diag_true = np.sqrt(((im64 - s64) ** 2).sum(1))          # [N] exact
    b_full = (MARGIN + diag_true).astype(np.float32)         # [N] exact fp32

    _f8 = mybir.dt.np(_P8)
    im_hi = im.astype(_f8)
    im_lo = (im - im_hi.astype(np.float32)).astype(_f8)
    s_hi = s.astype(_f8)
    s_lo = (s - s_hi.astype(np.float32)).astype(_f8)
    dot_terms = int(os.environ.get("DOT_TERMS", "2"))
    im_q64 = im_hi.astype(np.float64) + (
        im_lo.astype(np.float64) if dot_terms >= 2 else 0.0
    )
    s_q64 = s_hi.astype(np.float64) + (
        s_lo.astype(np.float64) if dot_terms >= 3 else 0.0
    )
    im_sq = (im_q64 ** 2).sum(1).astype(np.float32)                   # [N]
    s_sq = (s_q64 ** 2).sum(1)                                        # [N] f64

    # 4-term fp8 split of -s_sq/2 (residual splitting; |err| < 1e-3)
    foldv = -0.5 * s_sq
    frs, rem = [], foldv.copy()
    for _ in range(4):
        r = rem.astype(np.float32).astype(_f8)
        frs.append(r)
        rem = rem - r.astype(np.float64)

    ident = np.eye(128, dtype=np.float32).astype(BF16)
    nident = (np.eye(128, dtype=np.float32) * np.float32(-BIG)).astype(BF16)

    in_maps = []
    for c in range(NCORES):
        rows = slice(c * SLAB, (c + 1) * SLAB)
        rot = np.roll(np.arange(N), -c * SLAB)
        imTh = np.ascontiguousarray(im_hi[rows].T.reshape(2, 128, SLAB))
        imTl = np.ascontiguousarray(im_lo[rows].T.reshape(2, 128, SLAB))
        sTh = np.ascontiguousarray(s_hi[rot].T.reshape(2, 128, N))
        sTl = np.ascontiguousarray(s_lo[rot].T.reshape(2, 128, N))
        # DoubleRow K layout: k = p + 2*i for tile [p, i, :]
        foldc = np.ascontiguousarray(
            np.stack(
                [
                    np.stack([frs[0][rot], frs[2][rot]]),
                    np.stack([frs[1][rot], frs[3][rot]]),
                ]
            )
        )
        browc = np.ascontiguousarray(
            (b_full[rot] - np.float32(BSHIFT)).astype(BF16)
        )
        avecc = np.ascontiguousarray(b_full[rows].reshape(MT, 128).T)
        imsqc = np.ascontiguousarray(im_sq[rows].reshape(MT, 128).T)
        in_maps.append(
            {
                "imTh": imTh,
                "imTl": imTl,
                "sTh": sTh,
                "sTl": sTl,
                "fold": foldc,
                "brow": browc,
                "avec": avecc,
                "imsq": imsqc,
                "ident": ident,
                "nident": nident,
            }
        )
    return in_maps


_NC_CACHE = None


def get_module():
    global _NC_CACHE
    if _NC_CACHE is None:
        _NC_CACHE = build_module()
    return _NC_CACHE


def kernel(im: np.ndarray, s: np.ndarray) -> np.ndarray:
    nc = get_module()
    in_maps = prepare_inputs(im, s)
    res = bass_utils.run_bass_kernel_spmd(
        nc, in_maps, core_ids=list(range(NCORES))
    )
    total = 0.0
    for c in range(NCORES):
        total += float(res.results[c]["out"][0, 0])
    return np.array(np.float64(total) / (N * N), dtype=np.float32)

